# revision 1
# baseline (speedup 1.0000x reference)
"""Cross-attention Trainium2 Bass kernel.

Problem: B=4, Nq=Nk=1024, D=1024, H=16 heads, dh=64.
  Qn = LN(Q); Kn = LN(K)
  q = Qn@Wq.T; k = Kn@Wk.T; v = V@Wv.T   (per head dh=64)
  A = softmax(q.k / sqrt(1024))  (clip +-1e4 never triggers: |scores| < 1)
  O = LN(A@v); out = O + gelu(O@Wo.T)

Sharding: 8 cores = (batch b, query half). Core c handles queries
[half*512, half*512+512) of batch b = c//2. K/V projections for batch b are
computed on both of its cores (no collectives needed).

On-chip layout: everything transposed [feature, row] ("T-layout").
 - Host pre-transposes Q/K/V slices and weights (W.T = [d_in, d_out]) and
   pre-rounds all matmul inputs to fp32r (11-bit mantissa).
 - LN stats over the partition axis via ones-matmul (the [128,128] all-ones
   stationary operand makes the output already broadcast across partitions);
   LN(Q)/LN(K) fold into the projection evacuations:
   (x-m)r @ W = r*(x@W) + (-r*m)*colsum(W).
 - Softmax: per head pair, scoresT[j,i] via two adjacent K=64 matmuls packed
   into disjoint PE row groups; one 1024-wide exp per j-chunk. No max
   subtraction needed (|s| < 1). The softmax denominator S rides along the
   A@V matmul as a ones column at psum row 64+(h%8); S rows collect into two
   half-collectors (heads 8..15 / 0..7) for two batched reciprocals, so the
   normalization of the first half overlaps the second half's attention.
 - fp32r matmuls may only write PSUM starting at partition 0, and PSUM
   reads / matmul contraction rows must start 32-aligned; odd heads' A@V
   outputs are moved to partitions 64..127 with a shift-matrix matmul.
"""

import numpy as np

N_CORES = 8
D = 1024          # model dim (= Dq = Dv = Do)
IW = 512          # queries per core
NK = 1024         # keys
H = 16            # heads
DH = 64           # head dim
NCH = D // 128    # 8 partition chunks of the feature dim
SCALE = 1.0 / 32.0  # 1/sqrt(1024)
EPS = 1e-5
VW = 72           # v_sb columns per head: [v(64) | ones@64+(h%8) in pad(8)]

_CACHED_NC = None


def _round_fp32r(x):
    """Round fp32 to fp32r: 11-bit mantissa (round-to-nearest-even)."""
    u = np.ascontiguousarray(x, dtype=np.float32).view(np.uint32)
    rounded = (u + np.uint32(0x800) - ((u >> 12) & np.uint32(1))) & np.uint32(0xFFFFF000)
    return rounded.view(np.float32)


def _build_nc():
    import concourse.tile as tile
    import concourse.mybir as mybir
    from concourse import bacc

    f32 = mybir.dt.float32
    f32r = mybir.dt.float32r

    nc = bacc.Bacc("TRN2", target_bir_lowering=False, debug=False,
                   num_devices=N_CORES)

    def din(name, shape, dt=f32r):
        return nc.dram_tensor(name, shape, dt, kind="ExternalInput").ap()

    aps = dict(
        qt=din("qt", [D, IW]),          # Q.T slice  [d, i]
        kt=din("kt", [D, NK]),          # K.T        [d, j]
        vt=din("vt", [D, NK]),          # V.T        [d, j]
        wq=din("wq", [D, D]),           # Wq.T       [d_in, d_out]
        wk=din("wk", [D, D]),
        wv=din("wv", [D, D]),
        wo=din("wo", [D, D]),
        wks=din("wks", [D], f32),       # colsum of rounded Wk.T
        wqs=din("wqs", [D], f32),
        wos=din("wos", [D], f32),
        ones=din("ones", [128, 128]),
        shiftm=din("shiftm", [128, 128]),   # shift[k, 64+k] = 1
        bcm=din("bcm", [128, NCH, 128]),    # 1/S broadcast masks per chunk
        out=nc.dram_tensor("out", [D, IW], f32, kind="ExternalOutput").ap(),
    )

    with tile.TileContext(nc) as tc:
        _emit(tc, mybir, aps)
    nc.compile()
    return nc


def _emit(tc, mybir, aps):
    from contextlib import ExitStack
    from concourse.alu_op_type import AluOpType as Alu

    nc = tc.nc
    f32 = mybir.dt.float32
    f32r = mybir.dt.float32r
    AF = mybir.ActivationFunctionType

    ctx = ExitStack()
    with ctx:
        p_big = ctx.enter_context(tc.tile_pool(name="big", bufs=2))
        p_col = ctx.enter_context(tc.tile_pool(name="col", bufs=2))
        p_per = ctx.enter_context(tc.tile_pool(name="per", bufs=1))
        p_ln = ctx.enter_context(tc.tile_pool(name="ln", bufs=6))
        p_scr = ctx.enter_context(tc.tile_pool(name="scr", bufs=3))
        p_nm = ctx.enter_context(tc.tile_pool(name="nm", bufs=1))
        p_sq = ctx.enter_context(tc.tile_pool(name="sq", bufs=1))

        # ---- constants ----
        ones_sb = p_per.tile([128, 128], f32r, tag="ones")
        nc.sync.dma_start(ones_sb[:], aps["ones"][:])
        ones_bf = p_per.tile([128, 128], mybir.dt.bfloat16, tag="onesbf")
        nc.vector.tensor_copy(ones_bf[:], ones_sb[:].bitcast(f32))
        shiftm_sb = p_per.tile([128, 128], f32r, tag="shiftm")
        nc.sync.dma_start(shiftm_sb[:], aps["shiftm"][:])
        bcm_sb = p_per.tile([128, NCH, 128], f32r, tag="bcm")
        nc.sync.dma_start(bcm_sb[:], aps["bcm"][:])
        wks_sb = p_per.tile([128, NCH], f32, tag="wks")
        nc.sync.dma_start(wks_sb[:], aps["wks"].rearrange("(c p) -> p c", p=128))
        wqs_sb = p_per.tile([128, NCH], f32, tag="wqs")
        nc.sync.dma_start(wqs_sb[:], aps["wqs"].rearrange("(c p) -> p c", p=128))
        wos_sb = p_per.tile([128, NCH], f32, tag="wos")
        nc.sync.dma_start(wos_sb[:], aps["wos"].rearrange("(c p) -> p c", p=128))

        # ---- raw activations (T-layout: [128, chunk, row]) ----
        # qt (2MB) first so Q-stats start ASAP; kt streams behind it
        qt_sb = p_big.tile([128, NCH, IW], f32r, tag="big")
        for dc in range(NCH):
            nc.sync.dma_start(
                qt_sb[:, dc, :],
                aps["qt"].rearrange("(c p) i -> p c i", p=128)[:, dc, :])
        kt_sb = p_big.tile([128, NCH, NK], f32r, tag="big")
        for dc in range(NCH):
            nc.sync.dma_start(
                kt_sb[:, dc, :],
                aps["kt"].rearrange("(c p) j -> p c j", p=128)[:, dc, :])

        # persistent products
        kT = p_per.tile([128, NCH, NK], f32r, tag="kt")      # k.T [o, j]
        v_sb = p_per.tile([128, NCH, H * VW], f32r, tag="v")  # v [j, head-blk]
        qT = p_per.tile([128, NCH, IW], f32r, tag="qt")      # q.T [o, i]
        OT = p_per.tile([128, NCH, IW], f32r, tag="ot")      # attn out.T [o, i]
        coll_lo = p_per.tile([128, 512], f32, tag="cl")      # S heads 0..7
        coll_hi = p_per.tile([128, 512], f32, tag="ch")      # S heads 8..15
        collr_lo = p_per.tile([128, 512], f32r, tag="crl")   # 1/S
        collr_hi = p_per.tile([128, 512], f32r, tag="crh")

        # zero-fill the v pad region (cols 64..71 of each head block)
        nc.vector.tensor_copy(
            v_sb.rearrange("p c (h w) -> p c h w", w=VW)[:, :, :, DH:VW],
            nc.const_aps.tensor(0.0, (128, NCH, H, VW - DH)))

        def ln_stats(x_sb, jb, ps_pool, desc=False):
            """Partition-axis LN stats of x_sb[:, :, jb*512 : jb*512+512].
            Returns (r_bc, nB_bc): [128, 512] f32, broadcast on partitions;
            r = 1/std, nB = -mean/std."""
            sl = slice(jb * 512, jb * 512 + 512)
            ps_sum = ps_pool.tile([128, 512], f32, tag="stat", bufs=2)
            ps_sq = ps_pool.tile([128, 512], f32, tag="stat", bufs=2)
            order = range(NCH - 1, -1, -1) if desc else range(NCH)
            for n, dc in enumerate(order):
                sq = p_sq.tile([128, 512], mybir.dt.bfloat16, tag="sq")
                nc.scalar.activation(sq[:], x_sb[:, dc, sl], AF.Square)
                nc.tensor.matmul(ps_sum[:], ones_sb[:], x_sb[:, dc, sl],
                                 start=(n == 0), stop=(n == NCH - 1))
                nc.tensor.matmul(ps_sq[:], ones_bf[:], sq[:],
                                 start=(n == 0), stop=(n == NCH - 1))
            nm = p_nm.tile([128, 512], f32, tag="nm")     # -mean
            nc.scalar.activation(nm[:], ps_sum[:], AF.Copy, scale=-1.0 / D)
            q2 = p_scr.tile([128, 512], f32, tag="scr")   # E[x^2]
            nc.scalar.activation(q2[:], ps_sq[:], AF.Copy, scale=1.0 / D)
            msq = p_scr.tile([128, 512], f32, tag="scr")
            nc.vector.tensor_tensor(msq[:], nm[:], nm[:], Alu.mult)
            var = p_scr.tile([128, 512], f32, tag="scr")
            nc.vector.scalar_tensor_tensor(var[:], msq[:], -1.0, q2[:],
                                           Alu.mult, Alu.add)  # q2 - msq
            nc.vector.tensor_scalar_add(var[:], var[:], EPS)
            std = p_scr.tile([128, 512], f32, tag="scr")
            nc.scalar.activation(std[:], var[:], AF.Sqrt)
            r_bc = p_ln.tile([128, 512], f32, tag="ln")
            nc.vector.reciprocal(r_bc[:], std[:])
            nB_bc = p_ln.tile([128, 512], f32, tag="ln")
            nc.vector.tensor_tensor(nB_bc[:], nm[:], r_bc[:], Alu.mult)
            return r_bc, nB_bc

        with tc.tile_pool(name="ps1", bufs=1, space="PSUM") as ps1:
            # ---- PE warmup: keep the HAM activity window busy while the
            # first activation DMAs land (otherwise the first ~15us of real
            # matmuls run at the cold 1.2 GHz clock) ----
            ps_w = ps1.tile([128, 512], f32, tag="stat", bufs=2)
            NWARM = 120
            for w in range(NWARM):
                nc.tensor.matmul(ps_w[:, 0:128], ones_sb[:], ones_sb[:],
                                 start=(w == 0), stop=(w == NWARM - 1))
            wsink = p_scr.tile([128, 512], f32, tag="scr")
            nc.vector.tensor_copy(wsink[0:1, 0:8], ps_w[0:1, 0:8])

            # ---- LN stats for Q then K ----
            rq, nBq = ln_stats(qt_sb, 0, ps1)
            rk, nBk = [], []
            for jb in range(2):
                r_, b_ = ln_stats(kt_sb, jb, ps1)
                rk.append(r_)
                nBk.append(b_)

            # ---- q-proj ----
            for oc in range(NCH - 1, -1, -1):
                wqc = p_col.tile([128, NCH, 128], f32r, tag="col")
                nc.sync.dma_start(
                    wqc[:], aps["wq"][:, oc * 128:(oc + 1) * 128]
                    .rearrange("(c p) o -> p c o", p=128))
                ps_q = ps1.tile([128, 512], f32, tag="proj", bufs=2)
                for dc in range(NCH):
                    nc.tensor.matmul(ps_q[:], wqc[:, dc, :], qt_sb[:, dc, :],
                                     start=(dc == 0), stop=(dc == NCH - 1))
                dst = qT[:, oc, :]
                nc.vector.tensor_tensor(dst, ps_q[:], rq[:], Alu.mult)
                nc.vector.scalar_tensor_tensor(
                    dst, nBq[:], wqs_sb[:, oc, None], dst, Alu.mult, Alu.add)

            # ---- k-proj: kT[o,j] = r[j]*(WkT.T@KT)[o,j] + nB[j]*wks[o] ----
            # descending oc so attention pair 7 gets its chunk first
            for oc in range(NCH - 1, -1, -1):
                wkc = p_col.tile([128, NCH, 128], f32r, tag="col")
                nc.sync.dma_start(
                    wkc[:], aps["wk"][:, oc * 128:(oc + 1) * 128]
                    .rearrange("(c p) o -> p c o", p=128))
                for jb in range(2):
                    sl = slice(jb * 512, jb * 512 + 512)
                    ps_k = ps1.tile([128, 512], f32, tag="proj", bufs=2)
                    for dc in range(NCH):
                        nc.tensor.matmul(ps_k[:], wkc[:, dc, :],
                                         kt_sb[:, dc, sl],
                                         start=(dc == 0), stop=(dc == NCH - 1))
                    dst = kT[:, oc, sl]
                    nc.vector.tensor_tensor(dst, ps_k[:], rk[jb][:], Alu.mult)
                    nc.vector.scalar_tensor_tensor(
                        dst, nBk[jb][:], wks_sb[:, oc, None], dst,
                        Alu.mult, Alu.add)

            # ---- v-proj: v[j, o] = (VT.T @ WvT)[j, o] ----
            wv_sb = p_big.tile([128, NCH, D], f32r, tag="big")
            for dc in range(NCH):
                nc.sync.dma_start(
                    wv_sb[:, dc, :],
                    aps["wv"].rearrange("(c p) o -> p c o", p=128)[:, dc, :])
            for jc in range(NCH):
                vtc = p_col.tile([128, NCH, 128], f32r, tag="col")
                nc.sync.dma_start(
                    vtc[:], aps["vt"][:, jc * 128:(jc + 1) * 128]
                    .rearrange("(c p) j -> p c j", p=128))
                for ob in range(2):
                    sl = slice(ob * 512, ob * 512 + 512)
                    ps_v = ps1.tile([128, 512], f32, tag="proj", bufs=2)
                    for dc in range(NCH):
                        nc.tensor.matmul(ps_v[:], vtc[:, dc, :],
                                         wv_sb[:, dc, sl],
                                         start=(dc == 0), stop=(dc == NCH - 1))
                    # scatter 8 heads x 64 cols into VW-strided blocks
                    base = 8 * ob * VW
                    nc.vector.tensor_copy(
                        v_sb[:, jc, base:base + 8 * VW]
                        .rearrange("p (t w) -> p t w", w=VW)[:, :, 0:DH],
                        ps_v[:].rearrange("p (t w) -> p t w", w=DH))
            # ones column of head h at block offset 64+(h%8):
            # global positions 576*a + 64 + 73*t  (a = h//8, t = h%8)
            for a in range(2):
                nc.vector.tensor_copy(
                    v_sb[:, :, 576 * a + 64:576 * a + 576:73],
                    ones_sb[:, None, 0:8].to_broadcast((128, NCH, 8)))

        # ================= attention =================
        # Head pairs DESCENDING: two K=64 scores matmuls packed into disjoint
        # PE row groups; one 1024-wide exp per j-chunk (psum spans 2 banks).
        # S-row copies must read psum from partition 64, so head h copies rows
        # [64 : 65+h%8] (rows below its S are zero pads); descending order
        # means later copies never clobber collected S values.
        with tc.tile_pool(name="ps2", bufs=1, space="PSUM") as ps2:
            for pr in range(H // 2 - 1, -1, -1):
                hc = pr                     # feature chunk of this pair
                ET = p_big.tile([128, NCH, 1024], f32r, tag="big")
                for jc in range(NCH):
                    ps_s = ps2.tile([128, 1024], f32, tag="sc", bufs=2)
                    for hp in range(2):
                        prow = slice(hp * 64, hp * 64 + 64)
                        nc.tensor.matmul(
                            ps_s[:, hp * 512:hp * 512 + 512],
                            kT[prow, hc, jc * 128:(jc + 1) * 128],
                            qT[prow, hc, :], start=True, stop=True,
                            tile_position=(64 * hp, 0))
                    nc.scalar.activation(ET[:, jc, :], ps_s[:], AF.Exp,
                                         scale=SCALE)
                # A@V per head; ones col at row 64+(h%8) accumulates S
                for hp in (1, 0):
                    h = 2 * pr + hp
                    hm = h % 8
                    coll = coll_hi if h >= 8 else coll_lo
                    ps_o = ps2.tile([128, 512], f32, tag="av", bufs=2)
                    for jc in range(NCH):
                        nc.tensor.matmul(
                            ps_o[0:DH + 1 + hm, :],
                            v_sb[:, jc, h * VW:h * VW + DH + 1 + hm],
                            ET[:, jc, hp * 512:hp * 512 + 512],
                            start=(jc == 0), stop=(jc == NCH - 1))
                    nc.vector.tensor_copy(coll[64:65 + hm, :],
                                          ps_o[64:65 + hm, :])
                    if hp == 0:
                        nc.vector.tensor_copy(OT[0:64, hc, :], ps_o[0:64, :])
                    else:
                        tmp = p_scr.tile([128, 512], f32r, tag="scr")
                        nc.vector.tensor_copy(tmp[0:64, :], ps_o[0:64, :])
                        ps_sh = ps2.tile([128, 512], f32, tag="sh", bufs=1)
                        nc.tensor.matmul(ps_sh[:], shiftm_sb[0:64, :],
                                         tmp[0:64, :], start=True, stop=True)
                        nc.scalar.activation(OT[64:128, hc, :],
                                             ps_sh[64:128, :], AF.Copy)

        # ============ deferred softmax normalization + LN(O) + final ========
        with tc.tile_pool(name="ps3", bufs=1, space="PSUM") as ps3:
            with nc.allow_low_precision(reason="fp32r rhs for bc matmul"):
                nc.vector.reciprocal(collr_hi[64:72, :], coll_hi[64:72, :])
                nc.vector.reciprocal(collr_lo[64:72, :], coll_lo[64:72, :])
            for hc in range(NCH - 1, -1, -1):
                collr = collr_hi if hc >= 4 else collr_lo
                ps_b = ps3.tile([128, 512], f32, tag="bc", bufs=2)
                nc.tensor.matmul(ps_b[:], bcm_sb[64:72, hc, :],
                                 collr[64:72, :], start=True, stop=True,
                                 tile_position=(64, 0))
                sbc = p_scr.tile([128, 512], f32, tag="scr")
                nc.scalar.activation(sbc[:], ps_b[:], AF.Copy)
                nc.vector.tensor_tensor(OT[:, hc, :], OT[:, hc, :], sbc[:],
                                        Alu.mult)

            # LN(O) folds into the final-matmul evacuation exactly like
            # LN(Q)/LN(K): G = r[i]*(WoT.T@OT)[g,i] + nB[i]*wos[g], so the
            # final matmuls run on UNNORMALIZED (but 1/S-scaled) OT and only
            # the evacuation waits for the stats chain.
            ro, nBo = ln_stats(OT, 0, ps3, desc=True)
            fin = p_big.tile([128, NCH, IW], f32, tag="big")
            for gc in range(NCH):
                woc = p_col.tile([128, NCH, 128], f32r, tag="col")
                nc.sync.dma_start(
                    woc[:], aps["wo"][:, gc * 128:(gc + 1) * 128]
                    .rearrange("(c p) g -> p c g", p=128))
                ps_g = ps3.tile([128, 512], f32, tag="fing", bufs=2)
                for n, oc in enumerate(range(NCH - 1, -1, -1)):
                    nc.tensor.matmul(ps_g[:], woc[:, oc, :], OT[:, oc, :],
                                     start=(n == 0), stop=(n == NCH - 1))
                G = p_scr.tile([128, 512], f32, tag="scr")
                nc.vector.tensor_tensor(G[:], ps_g[:], ro[:], Alu.mult)
                nc.vector.scalar_tensor_tensor(
                    G[:], nBo[:], wos_sb[:, gc, None], G[:], Alu.mult, Alu.add)
                gel = p_scr.tile([128, 512], f32, tag="scr")
                nc.scalar.activation(gel[:], G[:], AF.Gelu)
                # residual LNO chunk = OT*r + nB (on GpSimd: it's idle here)
                res = p_scr.tile([128, 512], f32, tag="scr")
                nc.gpsimd.tensor_tensor(res[:], OT[:, gc, :].bitcast(f32),
                                        ro[:], Alu.mult)
                nc.gpsimd.tensor_tensor(res[:], res[:], nBo[:], Alu.add)
                nc.vector.tensor_tensor(fin[:, gc, :], gel[:], res[:], Alu.add)
                nc.sync.dma_start(
                    aps["out"].rearrange("(c p) i -> p c i", p=128)[:, gc, :],
                    fin[:, gc, :])


def _get_nc():
    global _CACHED_NC
    if _CACHED_NC is None:
        _CACHED_NC = _build_nc()
    return _CACHED_NC


def _prep_in_maps(inputs):
    Q, K, V = inputs["Q"], inputs["K"], inputs["V"]
    wq = _round_fp32r(np.ascontiguousarray(np.asarray(inputs["Wq"], np.float32).T))
    wk = _round_fp32r(np.ascontiguousarray(np.asarray(inputs["Wk"], np.float32).T))
    wv = _round_fp32r(np.ascontiguousarray(np.asarray(inputs["Wv"], np.float32).T))
    wo = _round_fp32r(np.ascontiguousarray(np.asarray(inputs["Wo"], np.float32).T))
    wks = wk.sum(axis=0, dtype=np.float32)
    wqs = wq.sum(axis=0, dtype=np.float32)
    wos = wo.sum(axis=0, dtype=np.float32)
    ones = np.ones((128, 128), np.float32)
    shiftm = np.zeros((128, 128), np.float32)
    shiftm[np.arange(64), 64 + np.arange(64)] = 1.0
    # bcm[64 + (2hc)%8, hc, 0:64] = 1 ; bcm[64 + (2hc+1)%8, hc, 64:128] = 1
    bcm = np.zeros((128, NCH, 128), np.float32)
    for hc in range(NCH):
        bcm[64 + (2 * hc) % 8, hc, 0:64] = 1.0
        bcm[64 + (2 * hc + 1) % 8, hc, 64:128] = 1.0
    in_maps = []
    for c in range(N_CORES):
        b, half = divmod(c, 2)
        qs = np.asarray(Q[b, half * IW:(half + 1) * IW, :], np.float32)
        in_maps.append({
            "qt": _round_fp32r(qs.T),
            "kt": _round_fp32r(np.asarray(K[b], np.float32).T),
            "vt": _round_fp32r(np.asarray(V[b], np.float32).T),
            "wq": wq, "wk": wk, "wv": wv, "wo": wo,
            "wks": wks, "wqs": wqs, "wos": wos, "ones": ones, "shiftm": shiftm,
            "bcm": bcm,
        })
    return in_maps


def run(inputs, trace=False):
    """Run the kernel; returns (output [4,1024,1024] f32, BassKernelResults)."""
    from concourse.bass_utils import run_bass_kernel_spmd
    nc = _get_nc()
    in_maps = _prep_in_maps(inputs)
    res = run_bass_kernel_spmd(nc, in_maps, core_ids=list(range(N_CORES)),
                               trace=trace)
    B = 4
    out = np.empty((B, 2 * IW, D), np.float32)
    for c in range(N_CORES):
        b, half = divmod(c, 2)
        out[b, half * IW:(half + 1) * IW, :] = res.results[c]["out"].T
    return out, res


def kernel(**inputs) -> np.ndarray:
    out, _ = run(inputs, trace=False)
    return out



# revision 20
# speedup vs baseline: 1.0284x; 1.0284x over previous
"""Cross-attention Trainium2 Bass kernel.

Problem: B=4, Nq=Nk=1024, D=1024, H=16 heads, dh=64.
  Qn = LN(Q); Kn = LN(K)
  q = Qn@Wq.T; k = Kn@Wk.T; v = V@Wv.T   (per head dh=64)
  A = softmax(q.k / sqrt(1024))  (clip +-1e4 never triggers: |scores| < 1)
  O = LN(A@v); out = O + gelu(O@Wo.T)

Sharding: 8 cores = (batch b, query half). Core c handles queries
[half*512, half*512+512) of batch b = c//2. K/V projections for batch b are
computed on both of its cores (no collectives needed).

On-chip layout: everything transposed [feature, row] ("T-layout").
 - Host pre-transposes Q/K/V slices and weights (W.T = [d_in, d_out]) and
   pre-rounds all matmul inputs to fp32r (11-bit mantissa).
 - LN stats over the partition axis via ones-matmul (the [128,128] all-ones
   stationary operand makes the output already broadcast across partitions);
   LN(Q)/LN(K) fold into the projection evacuations:
   (x-m)r @ W = r*(x@W) + (-r*m)*colsum(W).
 - Softmax: per head pair, scoresT[j,i] via two adjacent K=64 matmuls packed
   into disjoint PE row groups; one 1024-wide exp per j-chunk. No max
   subtraction needed (|s| < 1). The softmax denominator S rides along the
   A@V matmul as a ones column at psum row 64+(h%8); S rows collect into two
   half-collectors (heads 8..15 / 0..7) for two batched reciprocals, so the
   normalization of the first half overlaps the second half's attention.
 - fp32r matmuls may only write PSUM starting at partition 0, and PSUM
   reads / matmul contraction rows must start 32-aligned; odd heads' A@V
   outputs are moved to partitions 64..127 with a shift-matrix matmul.
"""

import numpy as np

N_CORES = 8
D = 1024          # model dim (= Dq = Dv = Do)
IW = 512          # queries per core
NK = 1024         # keys
H = 16            # heads
DH = 64           # head dim
NCH = D // 128    # 8 partition chunks of the feature dim
SCALE = 1.0 / 32.0  # 1/sqrt(1024)
EPS = 1e-5
VW = 72           # v_sb columns per head: [v(64) | ones@64+(h%8) in pad(8)]

_CACHED_NC = None
DEBUG = False


def _round_fp32r(x):
    """Round fp32 to fp32r: 11-bit mantissa (round-to-nearest-even)."""
    u = np.ascontiguousarray(x, dtype=np.float32).view(np.uint32)
    rounded = (u + np.uint32(0x800) - ((u >> 12) & np.uint32(1))) & np.uint32(0xFFFFF000)
    return rounded.view(np.float32)


def _build_nc():
    import concourse.tile as tile
    import concourse.mybir as mybir
    from concourse import bacc

    f32 = mybir.dt.float32
    f32r = mybir.dt.float32r

    nc = bacc.Bacc("TRN2", target_bir_lowering=False, debug=False,
                   num_devices=N_CORES)

    def din(name, shape, dt=f32r):
        return nc.dram_tensor(name, shape, dt, kind="ExternalInput").ap()

    aps = dict(
        qt=din("qt", [D, IW]),          # Q.T slice  [d, i]
        kt=din("kt", [D, NK]),          # K.T        [d, j]
        vt=din("vt", [D, NK]),          # V.T        [d, j]
        wq=din("wq", [D, D]),           # Wq.T       [d_in, d_out]
        wk=din("wk", [D, D]),
        wv=din("wv", [D, D]),
        wo=din("wo", [D, D]),
        wks=din("wks", [D], f32),       # colsum of rounded Wk.T
        wqs=din("wqs", [D], f32),
        ones=din("ones", [128, 128]),
        bcm=din("bcm", [128, NCH, 128]),    # 1/S broadcast masks per chunk
        out=nc.dram_tensor("out", [D, IW], f32, kind="ExternalOutput").ap(),
    )
    if DEBUG:
        for nm, shp in [("dbg_qt", [128, NCH, IW]), ("dbg_kt", [128, NCH, NK]),
                        ("dbg_ot1", [128, NCH, IW]), ("dbg_ot2", [128, NCH, IW]),
                        ("dbg_collr", [128, 512]), ("dbg_lno", [128, NCH, IW])]:
            aps[nm] = nc.dram_tensor(nm, shp, f32,
                                     kind="ExternalOutput").ap()

    with tile.TileContext(nc) as tc:
        _emit(tc, mybir, aps)
    nc.compile()
    return nc


def _emit(tc, mybir, aps):
    from contextlib import ExitStack
    from concourse.alu_op_type import AluOpType as Alu

    nc = tc.nc
    f32 = mybir.dt.float32
    f32r = mybir.dt.float32r
    AF = mybir.ActivationFunctionType

    ctx = ExitStack()
    with ctx:
        p_big = ctx.enter_context(tc.tile_pool(name="big", bufs=2))
        p_col = ctx.enter_context(tc.tile_pool(name="col", bufs=2))
        p_per = ctx.enter_context(tc.tile_pool(name="per", bufs=1))
        p_ln = ctx.enter_context(tc.tile_pool(name="ln", bufs=6))
        p_scr = ctx.enter_context(tc.tile_pool(name="scr", bufs=3))
        p_nm = ctx.enter_context(tc.tile_pool(name="nm", bufs=1))
        p_sq = ctx.enter_context(tc.tile_pool(name="sq", bufs=1))

        # ---- constants (ones first: warmup + stats need it) ----
        ones_sb = p_per.tile([128, 128], f32r, tag="ones")
        nc.sync.dma_start(ones_sb[:], aps["ones"][:])
        ones_bf = p_per.tile([128, 128], mybir.dt.bfloat16, tag="onesbf")
        nc.vector.tensor_copy(ones_bf[:], ones_sb[:].bitcast(f32))

        # ---- raw activations (T-layout: [128, chunk, row]) ----
        # qt (2MB) first so Q-stats start ASAP; kt streams behind it
        qt_sb = p_big.tile([128, NCH, IW], f32r, tag="big")
        for dc in range(NCH):
            nc.sync.dma_start(
                qt_sb[:, dc, :],
                aps["qt"].rearrange("(c p) i -> p c i", p=128)[:, dc, :])
        kt_sb = p_big.tile([128, NCH, NK], f32r, tag="big")
        for dc in range(NCH):
            nc.sync.dma_start(
                kt_sb[:, dc, :],
                aps["kt"].rearrange("(c p) j -> p c j", p=128)[:, dc, :])

        # small constants behind the activation streams
        wks_sb = p_per.tile([128, NCH], f32, tag="wks")
        nc.sync.dma_start(wks_sb[:], aps["wks"].rearrange("(c p) -> p c", p=128))
        wqs_sb = p_per.tile([128, NCH], f32, tag="wqs")
        nc.sync.dma_start(wqs_sb[:], aps["wqs"].rearrange("(c p) -> p c", p=128))
        bcm_sb = p_per.tile([128, NCH, 128], f32r, tag="bcm")
        nc.sync.dma_start(bcm_sb[:], aps["bcm"][:])

        # persistent products
        kT = p_per.tile([128, NCH, NK], f32r, tag="kt")      # k.T [o, j]
        v_sb = p_per.tile([128, NCH, H * VW], f32r, tag="v")  # v [j, head-blk]
        qT = p_per.tile([128, NCH, IW], f32r, tag="qt")      # q.T [o, i]
        OT = p_per.tile([128, NCH, IW], f32r, tag="ot")      # attn out.T [o, i]
        coll_lo = p_per.tile([128, 512], f32, tag="cl")      # S heads 0..7
        coll_hi = p_per.tile([128, 512], f32, tag="ch")      # S heads 8..15
        collr_lo = p_per.tile([128, 512], f32r, tag="crl")   # 1/S
        collr_hi = p_per.tile([128, 512], f32r, tag="crh")

        # zero-fill the v pad region (cols 64..71 of each head block)
        nc.vector.tensor_copy(
            v_sb.rearrange("p c (h w) -> p c h w", w=VW)[:, :, :, DH:VW],
            nc.const_aps.tensor(0.0, (128, NCH, H, VW - DH)))

        def ln_stats(x_sb, jb, ps_pool, desc=False):
            """Partition-axis LN stats of x_sb[:, :, jb*512 : jb*512+512].
            Returns (r_bc, nB_bc): [128, 512] f32, broadcast on partitions;
            r = 1/std, nB = -mean/std."""
            sl = slice(jb * 512, jb * 512 + 512)
            ps_sum = ps_pool.tile([128, 512], f32, tag="stat", bufs=2)
            ps_sq = ps_pool.tile([128, 512], f32, tag="stat", bufs=2)
            order = range(NCH - 1, -1, -1) if desc else range(NCH)
            for n, dc in enumerate(order):
                sq = p_sq.tile([128, 512], mybir.dt.bfloat16, tag="sq")
                nc.scalar.activation(sq[:], x_sb[:, dc, sl], AF.Square)
                nc.tensor.matmul(ps_sum[:], ones_sb[:], x_sb[:, dc, sl],
                                 start=(n == 0), stop=(n == NCH - 1))
                nc.tensor.matmul(ps_sq[:], ones_bf[:], sq[:],
                                 start=(n == 0), stop=(n == NCH - 1))
            nm = p_nm.tile([128, 512], f32, tag="nm")     # -mean
            nc.scalar.activation(nm[:], ps_sum[:], AF.Copy, scale=-1.0 / D)
            q2 = p_scr.tile([128, 512], f32, tag="scr")   # E[x^2]
            nc.scalar.activation(q2[:], ps_sq[:], AF.Copy, scale=1.0 / D)
            msq = p_scr.tile([128, 512], f32, tag="scr")
            nc.vector.tensor_tensor(msq[:], nm[:], nm[:], Alu.mult)
            var = p_scr.tile([128, 512], f32, tag="scr")
            nc.vector.scalar_tensor_tensor(var[:], msq[:], -1.0, q2[:],
                                           Alu.mult, Alu.add)  # q2 - msq
            nc.vector.tensor_scalar_add(var[:], var[:], EPS)
            std = p_scr.tile([128, 512], f32, tag="scr")
            nc.scalar.activation(std[:], var[:], AF.Sqrt)
            r_bc = p_ln.tile([128, 512], f32, tag="ln")
            nc.vector.reciprocal_approx_fast(r_bc[:], std[:])
            nB_bc = p_ln.tile([128, 512], f32, tag="ln")
            nc.vector.tensor_tensor(nB_bc[:], nm[:], r_bc[:], Alu.mult)
            return r_bc, nB_bc

        with tc.tile_pool(name="ps1", bufs=1, space="PSUM") as ps1:
            # ---- PE warmup: keep the HAM activity window busy while the
            # first activation DMAs land (otherwise the first ~15us of real
            # matmuls run at the cold 1.2 GHz clock) ----
            ps_w = ps1.tile([128, 512], f32, tag="stat", bufs=2)
            NWARM = 40
            for w in range(NWARM):
                nc.tensor.matmul(ps_w[:, 0:128], ones_sb[:], ones_sb[:],
                                 start=(w == 0), stop=(w == NWARM - 1))
            wsink = p_scr.tile([128, 512], f32, tag="scr")
            nc.vector.tensor_copy(wsink[0:1, 0:8], ps_w[0:1, 0:8])

            # ---- LN stats for Q then K ----
            rq, nBq = ln_stats(qt_sb, 0, ps1)
            rk, nBk = [], []
            for jb in range(2):
                r_, b_ = ln_stats(kt_sb, jb, ps1)
                rk.append(r_)
                nBk.append(b_)

            # ---- q-proj ----
            for oc in range(NCH - 1, -1, -1):
                wqc = p_col.tile([128, NCH, 128], f32r, tag="col")
                nc.sync.dma_start(
                    wqc[:], aps["wq"][:, oc * 128:(oc + 1) * 128]
                    .rearrange("(c p) o -> p c o", p=128))
                ps_q = ps1.tile([128, 512], f32, tag="proj", bufs=2)
                for dc in range(NCH):
                    nc.tensor.matmul(ps_q[:], wqc[:, dc, :], qt_sb[:, dc, :],
                                     start=(dc == 0), stop=(dc == NCH - 1))
                dst = qT[:, oc, :]
                nc.vector.tensor_tensor(dst, ps_q[:], rq[:], Alu.mult)
                nc.vector.scalar_tensor_tensor(
                    dst, nBq[:], wqs_sb[:, oc, None], dst, Alu.mult, Alu.add)

            # ---- k-proj: kT[o,j] = r[j]*(WkT.T@KT)[o,j] + nB[j]*wks[o] ----
            # descending oc so attention pair 7 gets its chunk first
            for oc in range(NCH - 1, -1, -1):
                wkc = p_col.tile([128, NCH, 128], f32r, tag="col")
                nc.sync.dma_start(
                    wkc[:], aps["wk"][:, oc * 128:(oc + 1) * 128]
                    .rearrange("(c p) o -> p c o", p=128))
                for jb in range(2):
                    sl = slice(jb * 512, jb * 512 + 512)
                    ps_k = ps1.tile([128, 512], f32, tag="proj", bufs=2)
                    for dc in range(NCH):
                        nc.tensor.matmul(ps_k[:], wkc[:, dc, :],
                                         kt_sb[:, dc, sl],
                                         start=(dc == 0), stop=(dc == NCH - 1))
                    dst = kT[:, oc, sl]
                    nc.vector.tensor_tensor(dst, ps_k[:], rk[jb][:], Alu.mult)
                    nc.vector.scalar_tensor_tensor(
                        dst, nBk[jb][:], wks_sb[:, oc, None], dst,
                        Alu.mult, Alu.add)

            # ---- v-proj: v[j, o] = (VT.T @ WvT)[j, o] ----
            wv_sb = p_big.tile([128, NCH, D], f32r, tag="big")
            for dc in range(NCH):
                nc.sync.dma_start(
                    wv_sb[:, dc, :],
                    aps["wv"].rearrange("(c p) o -> p c o", p=128)[:, dc, :])
            for jc in range(NCH):
                vtc = p_col.tile([128, NCH, 128], f32r, tag="col")
                nc.sync.dma_start(
                    vtc[:], aps["vt"][:, jc * 128:(jc + 1) * 128]
                    .rearrange("(c p) j -> p c j", p=128))
                for ob in range(2):
                    sl = slice(ob * 512, ob * 512 + 512)
                    ps_v = ps1.tile([128, 512], f32, tag="proj", bufs=2)
                    for dc in range(NCH):
                        nc.tensor.matmul(ps_v[:], vtc[:, dc, :],
                                         wv_sb[:, dc, sl],
                                         start=(dc == 0), stop=(dc == NCH - 1))
                    # scatter 8 heads x 64 cols into VW-strided blocks
                    base = 8 * ob * VW
                    nc.vector.tensor_copy(
                        v_sb[:, jc, base:base + 8 * VW]
                        .rearrange("p (t w) -> p t w", w=VW)[:, :, 0:DH],
                        ps_v[:].rearrange("p (t w) -> p t w", w=DH))
            # ones column of head h at block offset 64+(h%8):
            # global positions 576*a + 64 + 73*t  (a = h//8, t = h%8)
            for a in range(2):
                nc.vector.tensor_copy(
                    v_sb[:, :, 576 * a + 64:576 * a + 576:73],
                    ones_sb[:, None, 0:8].to_broadcast((128, NCH, 8)))

        if "dbg_qt" in aps:
            nc.sync.dma_start(aps["dbg_qt"][:], qT[:].bitcast(f32))
            nc.sync.dma_start(aps["dbg_kt"][:], kT[:].bitcast(f32))

        # ================= attention =================
        # Head pairs DESCENDING: two K=64 scores matmuls packed into disjoint
        # PE row groups; one 1024-wide exp per j-chunk (psum spans 2 banks).
        # S-row copies must read psum from partition 64, so head h copies rows
        # [64 : 65+h%8] (rows below its S are zero pads); descending order
        # means later copies never clobber collected S values.
        with tc.tile_pool(name="ps2", bufs=1, space="PSUM") as ps2:
            for pr in range(H // 2 - 1, -1, -1):
                hc = pr                     # feature chunk of this pair
                ET = p_big.tile([128, NCH, 1024], f32r, tag="big")
                for jc in range(NCH):
                    ps_s = ps2.tile([128, 1024], f32, tag="sc", bufs=2)
                    for hp in range(2):
                        prow = slice(hp * 64, hp * 64 + 64)
                        nc.tensor.matmul(
                            ps_s[:, hp * 512:hp * 512 + 512],
                            kT[prow, hc, jc * 128:(jc + 1) * 128],
                            qT[prow, hc, :], start=True, stop=True,
                            tile_position=(64 * hp, 0))
                    nc.scalar.activation(ET[:, jc, :], ps_s[:], AF.Exp,
                                         scale=SCALE)
                # A@V per head; ones col at row 64+(h%8) accumulates S
                for hp in (1, 0):
                    h = 2 * pr + hp
                    hm = h % 8
                    coll = coll_hi if h >= 8 else coll_lo
                    ps_o = ps2.tile([128, 512], f32, tag="av", bufs=2)
                    for jc in range(NCH):
                        nc.tensor.matmul(
                            ps_o[0:DH + 1 + hm, :],
                            v_sb[:, jc, h * VW:h * VW + DH + 1 + hm],
                            ET[:, jc, hp * 512:hp * 512 + 512],
                            start=(jc == 0), stop=(jc == NCH - 1))
                    nc.vector.tensor_copy(coll[64:65 + hm, :],
                                          ps_o[64:65 + hm, :])
                    if hp == 0:
                        nc.vector.tensor_copy(OT[0:64, hc, :], ps_o[0:64, :])
                    else:
                        # odd head lands at psum partitions 0..63; move to OT
                        # partitions 64..127 with a partition-crossing
                        # SBUF->SBUF DMA (PE/ACT stay free for real work)
                        tmp = p_scr.tile([128, 512], f32r, tag="scr")
                        nc.vector.tensor_copy(tmp[0:64, :], ps_o[0:64, :])
                        nc.sync.dma_start(OT[64:128, hc, :], tmp[0:64, :])

        if "dbg_ot1" in aps:
            nc.sync.dma_start(aps["dbg_ot1"][:], OT[:].bitcast(f32))

        # ============ deferred softmax normalization + LN(O) + final ========
        with tc.tile_pool(name="ps3", bufs=1, space="PSUM") as ps3:
            with nc.allow_low_precision(reason="fp32r rhs for bc matmul"):
                nc.vector.reciprocal(collr_hi[64:72, :], coll_hi[64:72, :])
                nc.vector.reciprocal(collr_lo[64:72, :], coll_lo[64:72, :])
            for hc in range(NCH - 1, -1, -1):
                collr = collr_hi if hc >= 4 else collr_lo
                ps_b = ps3.tile([128, 512], f32, tag="bc", bufs=2)
                nc.tensor.matmul(ps_b[:], bcm_sb[64:72, hc, :],
                                 collr[64:72, :], start=True, stop=True,
                                 tile_position=(64, 0))
                nc.vector.tensor_tensor(OT[:, hc, :], OT[:, hc, :], ps_b[:],
                                        Alu.mult)

            if "dbg_ot2" in aps:
                nc.sync.dma_start(aps["dbg_ot2"][:], OT[:].bitcast(f32))
                nc.sync.dma_start(aps["dbg_collr"][:], collr_hi[:].bitcast(f32))

            # LN(O) materialized explicitly: LNO = OT*r + nB; the final
            # matmuls then contract LNO directly, gelu reads psum, and the
            # residual is just LNO.
            ro, nBo = ln_stats(OT, 0, ps3, desc=True)
            LNO = p_big.tile([128, NCH, IW], f32r, tag="big")
            for oc in range(NCH - 1, -1, -1):
                nc.vector.tensor_tensor(LNO[:, oc, :], OT[:, oc, :],
                                        ro[:], Alu.mult)
                nc.vector.tensor_tensor(LNO[:, oc, :], LNO[:, oc, :],
                                        nBo[:], Alu.add)
            if "dbg_lno" in aps:
                nc.sync.dma_start(aps["dbg_lno"][:], LNO[:].bitcast(f32))
            fin = p_big.tile([128, NCH, IW], f32, tag="big")
            for gc in range(NCH):
                woc = p_col.tile([128, NCH, 128], f32r, tag="col")
                nc.sync.dma_start(
                    woc[:], aps["wo"][:, gc * 128:(gc + 1) * 128]
                    .rearrange("(c p) g -> p c g", p=128))
                ps_g = ps3.tile([128, 512], f32, tag="fing", bufs=2)
                for n, oc in enumerate(range(NCH - 1, -1, -1)):
                    nc.tensor.matmul(ps_g[:], woc[:, oc, :], LNO[:, oc, :],
                                     start=(n == 0), stop=(n == NCH - 1))
                gel = p_scr.tile([128, 512], f32, tag="scr")
                nc.scalar.activation(gel[:], ps_g[:], AF.Gelu)
                nc.vector.tensor_tensor(fin[:, gc, :], gel[:],
                                        LNO[:, gc, :], Alu.add)
                nc.sync.dma_start(
                    aps["out"].rearrange("(c p) i -> p c i", p=128)[:, gc, :],
                    fin[:, gc, :])


def _get_nc():
    global _CACHED_NC
    if _CACHED_NC is None:
        _CACHED_NC = _build_nc()
    return _CACHED_NC


def _prep_in_maps(inputs):
    Q, K, V = inputs["Q"], inputs["K"], inputs["V"]
    wq = _round_fp32r(np.ascontiguousarray(np.asarray(inputs["Wq"], np.float32).T))
    wk = _round_fp32r(np.ascontiguousarray(np.asarray(inputs["Wk"], np.float32).T))
    wv = _round_fp32r(np.ascontiguousarray(np.asarray(inputs["Wv"], np.float32).T))
    wo = _round_fp32r(np.ascontiguousarray(np.asarray(inputs["Wo"], np.float32).T))
    wks = wk.sum(axis=0, dtype=np.float32)
    wqs = wq.sum(axis=0, dtype=np.float32)
    ones = np.ones((128, 128), np.float32)
    # bcm[64 + (2hc)%8, hc, 0:64] = 1 ; bcm[64 + (2hc+1)%8, hc, 64:128] = 1
    bcm = np.zeros((128, NCH, 128), np.float32)
    for hc in range(NCH):
        bcm[64 + (2 * hc) % 8, hc, 0:64] = 1.0
        bcm[64 + (2 * hc + 1) % 8, hc, 64:128] = 1.0
    in_maps = []
    for c in range(N_CORES):
        b, half = divmod(c, 2)
        qs = np.asarray(Q[b, half * IW:(half + 1) * IW, :], np.float32)
        in_maps.append({
            "qt": _round_fp32r(qs.T),
            "kt": _round_fp32r(np.asarray(K[b], np.float32).T),
            "vt": _round_fp32r(np.asarray(V[b], np.float32).T),
            "wq": wq, "wk": wk, "wv": wv, "wo": wo,
            "wks": wks, "wqs": wqs, "ones": ones,
            "bcm": bcm,
        })
    return in_maps


def run(inputs, trace=False):
    """Run the kernel; returns (output [4,1024,1024] f32, BassKernelResults)."""
    from concourse.bass_utils import run_bass_kernel_spmd
    nc = _get_nc()
    in_maps = _prep_in_maps(inputs)
    res = run_bass_kernel_spmd(nc, in_maps, core_ids=list(range(N_CORES)),
                               trace=trace)
    B = 4
    out = np.empty((B, 2 * IW, D), np.float32)
    for c in range(N_CORES):
        b, half = divmod(c, 2)
        out[b, half * IW:(half + 1) * IW, :] = res.results[c]["out"].T
    return out, res


def kernel(**inputs) -> np.ndarray:
    out, _ = run(inputs, trace=False)
    return out

